# revision 1
# baseline (speedup 1.0000x reference)
"""CenterLoss on 8 Trainium2 NeuronCores.

Math: the reference masks the full (B, C) distance matrix down to one entry
per row and clips zeros up to 1e-12, so

    loss = mean_b ||x_b - centers[labels_b]||^2 + (C-1) * 1e-12

exactly (the matched entries are chi-square-distributed around 4096 and never
touch either clip bound). No (B, C) matmul is needed — the kernel is a
row gather + fused subtract/square/reduce.

Distribution: data-parallel over the batch. Each of the 8 cores gets 1024
rows of x (negated, bf16) + their labels; centers (bf16) are replicated in
each core's DRAM. Per 128-row tile the kernel
  1. DMAs the -x tile to SBUF (HWDGE),
  2. indirect-DMA-gathers centers[label] on top of it with cce_op=add,
     leaving (c - x) in the tile (sign is irrelevant under squaring),
  3. runs one ScalarE activation Square with accum_out to get the
     per-partition sum of squares.
The 128x8 per-partition partials are summed on host in float64.

bf16 input rounding perturbs the scalar loss by ~2e-6 relative (errors on
16.8M squared terms average out); the host-side final sum is exact.
"""

import numpy as np

B = 8192
F = 2048
C = 4096
N_CORES = 8
P = 128
ROWS_PER_CORE = B // N_CORES  # 1024
TILES = ROWS_PER_CORE // P  # 8

_CACHE: dict = {}


def _build_program():
    import concourse.bacc as bacc
    import concourse.bass as bass
    import concourse.mybir as mybir
    from concourse.tile import TileContext

    nc = bacc.Bacc("TRN2", target_bir_lowering=False, debug=False,
                   num_devices=N_CORES)
    x = nc.dram_tensor("x", [ROWS_PER_CORE, F], mybir.dt.bfloat16,
                       kind="ExternalInput")  # holds -x, bf16
    labels_t = nc.dram_tensor("labels_t", [P, TILES], mybir.dt.int32,
                              kind="ExternalInput")  # [p, n] = label[n*128+p]
    centers = nc.dram_tensor("centers", [C, F], mybir.dt.bfloat16,
                             kind="ExternalInput")
    partials = nc.dram_tensor("partials", [P, TILES], mybir.dt.float32,
                              kind="ExternalOutput")

    x_tiles = x[:].rearrange("(n p) f -> n p f", p=P)

    with TileContext(nc) as tc:
        with (
            tc.tile_pool(name="work", bufs=4) as work,
            tc.tile_pool(name="small", bufs=1) as small,
        ):
            lab = small.tile([P, TILES], mybir.dt.int32)
            nc.sync.dma_start(out=lab[:], in_=labels_t[:])
            acc = small.tile([P, TILES], mybir.dt.float32)
            for n in range(TILES):
                t = work.tile([P, F], mybir.dt.bfloat16, tag="xt")
                nc.sync.dma_start(out=t[:], in_=x_tiles[n])
                # t += centers[labels] done by the DMA's inline CCE adder;
                # t ends up holding (c - x) rowwise.
                nc.gpsimd.indirect_dma_start(
                    out=t[:],
                    out_offset=None,
                    in_=centers[:],
                    in_offset=bass.IndirectOffsetOnAxis(ap=lab[:, n:n + 1],
                                                        axis=0),
                    compute_op=mybir.AluOpType.add,
                )
                nc.scalar.activation(
                    out=t[:], in_=t[:],
                    func=mybir.ActivationFunctionType.Square,
                    accum_out=acc[:, n:n + 1],
                )
            nc.sync.dma_start(out=partials[:], in_=acc[:])

    nc.compile()
    return nc


def _get_program():
    if "nc" not in _CACHE:
        _CACHE["nc"] = _build_program()
    return _CACHE["nc"]


def kernel(x, labels, centers, _trace=False, _trace_cores=None):
    import ml_dtypes
    from concourse.bass_utils import run_bass_kernel_spmd

    x = np.asarray(x)
    labels = np.asarray(labels)
    centers = np.asarray(centers)
    assert x.shape == (B, F) and centers.shape == (C, F)

    nc = _get_program()

    neg_x = np.ascontiguousarray((-x).astype(ml_dtypes.bfloat16))
    centers_b = np.ascontiguousarray(centers.astype(ml_dtypes.bfloat16))
    labels32 = labels.astype(np.int32)

    in_maps = []
    for k in range(N_CORES):
        lo = k * ROWS_PER_CORE
        lab_k = labels32[lo:lo + ROWS_PER_CORE].reshape(TILES, P).T
        in_maps.append({
            "x": neg_x[lo:lo + ROWS_PER_CORE],
            "labels_t": np.ascontiguousarray(lab_k),
            "centers": centers_b,
        })

    res = run_bass_kernel_spmd(
        nc, in_maps, list(range(N_CORES)),
        trace=_trace,
        trace_cores=_trace_cores if _trace else None,
    )
    _CACHE["last_result"] = res

    total = np.float64(0.0)
    for r in res.results:
        total += r["partials"].astype(np.float64).sum()
    loss = total / B + (C - 1) * 1e-12
    return np.float32(loss)


# revision 17
# speedup vs baseline: 1.1766x; 1.1766x over previous
"""CenterLoss on 8 Trainium2 NeuronCores.

Math: the reference masks the full (B, C) distance matrix down to one entry
per row and clips zeros up to 1e-12, so

    loss = mean_b ||x_b - centers[labels_b]||^2 + (C-1) * 1e-12

exactly (the matched entries are chi-square-distributed around 4096 and never
touch either clip bound). No (B, C) matmul is needed — the kernel is a
row gather + fused subtract/square/reduce.

Distribution: data-parallel over the batch. Each of the 8 cores gets 1024
rows of x (negated, bf16) + labels; centers (fp8-e3m4 by default) are
replicated in each core's DRAM. Per 128-row tile the kernel
  1. DMAs the -x tile to SBUF (HWDGE),
  2. indirect-DMA-gathers centers[label] onto it with cce_op=add — the
     DMA's inline CCE adder computes (c - x) in bf16; the sign is
     irrelevant under squaring,
  3. runs one ScalarE activation(Square, accum_out) for the row sums.
Per-partition partials are summed on host in float64.

Quantization handling: with c~ = q(c), x~ = bf16(x),
  ||x~ - c~||^2 - ||x - c||^2
    = [||dc||^2 + 2 dc.c] + [||dx||^2 + 2 dx.x] - 2 dx.c - 2 dc.x - 2 dx.dc
The bracketed self-terms are computed exactly on host (per-class for c,
per-row for x) and subtracted; the remaining cross terms are zero-mean
(quantization noise independent of the other operand) and contribute only
~4e-6 relative noise across the 16.8M summed elements.
"""

import numpy as np

B = 8192
F = 2048
C = 4096
N_CORES = 8
P = 128
ROWS_PER_CORE = B // N_CORES  # 1024
ROW_GROUPS = ROWS_PER_CORE // P  # 8

# --- tunables -------------------------------------------------------------
GATHER_MODE = "indirect"  # "indirect" (CCE-fused subtract) | "dma_gather"
CENTER_DT = "fp8e3"  # "bf16" | "fp8e3" | "fp8e4"
X_DT = "bf16"        # "bf16" | "fp8e3"
X_BUFS = 8
DVE_SQ_TILES = ()    # row-groups whose square+reduce runs on VectorE
N_GATHERS = 4        # dma_gather mode: gather ops per core
DMA_SCRATCH = 65536  # SWDGE descriptor-ring bytes (default 16384)
# --------------------------------------------------------------------------

_CACHE: dict = {}


def _np_dt(name):
    import ml_dtypes
    return {"bf16": ml_dtypes.bfloat16,
            "fp8e3": ml_dtypes.float8_e3m4,
            "fp8e4": ml_dtypes.float8_e4m3}[name]


def _build_program(mode, center_dt, x_dt_name, x_bufs, dve_sq, n_gathers, scratch):
    import concourse.bacc as bacc
    import concourse.bass as bass
    import concourse.mybir as mybir
    from concourse.tile import TileContext

    c_dt = {"bf16": mybir.dt.bfloat16,
            "fp8e3": mybir.dt.float8e3,
            "fp8e4": mybir.dt.float8e4}[center_dt]
    x_dt = {"bf16": mybir.dt.bfloat16,
            "fp8e3": mybir.dt.float8e3}[x_dt_name]

    nc = bacc.Bacc("TRN2", target_bir_lowering=False, debug=False,
                   num_devices=N_CORES, dynamic_dma_scratch_size=scratch,
                   num_swdge_queues=2)
    x = nc.dram_tensor("x", [ROWS_PER_CORE, F], x_dt,
                       kind="ExternalInput")  # holds -x
    labels_t = nc.dram_tensor("labels_t", [P, ROW_GROUPS], mybir.dt.int32,
                              kind="ExternalInput")  # [p, n] = label[n*128+p]
    # dma_gather mode: [p, s] = labels[s*16 + (p%16)], the 16-partition wrap
    # replicated into all 8 gpsimd cores' partition windows.
    idx16 = nc.dram_tensor("idx16", [P, ROWS_PER_CORE // 16], mybir.dt.int16,
                           kind="ExternalInput")
    centers = nc.dram_tensor("centers", [C, F], c_dt, kind="ExternalInput")
    partials = nc.dram_tensor("partials", [P, ROW_GROUPS], mybir.dt.float32,
                              kind="ExternalOutput")

    x_tiles = x[:].rearrange("(n p) f -> n p f", p=P)

    if mode == "dma_gather":
        return _build_dma_gather(nc, bass, mybir, TileContext, c_dt, x, idx16,
                                 centers, partials, x_tiles, x_bufs, dve_sq,
                                 n_gathers)
    assert x_dt_name == "bf16" or mode == "indirect"

    with TileContext(nc) as tc:
        with (
            tc.tile_pool(name="work", bufs=x_bufs) as work,
            tc.tile_pool(name="small", bufs=1) as small,
        ):
            # SWDGE load: precedes the gathers in the Q7 queue and keeps
            # their wait off the shared HWDGE sem lanes (an HWDGE labels
            # load shares a lane with the 8th x load and stalls gather 0).
            lab = small.tile([P, ROW_GROUPS], mybir.dt.int32)
            nc.gpsimd.dma_start(out=lab[:], in_=labels_t[:])
            acc = small.tile([P, ROW_GROUPS], mybir.dt.float32)
            junk = small.tile([P, F], mybir.dt.bfloat16)
            for n in range(ROW_GROUPS):
                tl = work.tile([P, F], x_dt, tag="xt")
                nc.sync.dma_start(out=tl[:], in_=x_tiles[n])
                # tl += centers[labels] via the DMA's inline CCE adder
                # (fp8 source is cast on the way); tl becomes (c - x).
                nc.gpsimd.indirect_dma_start(
                    out=tl[:],
                    out_offset=None,
                    in_=centers[:],
                    in_offset=bass.IndirectOffsetOnAxis(ap=lab[:, n:n + 1],
                                                        axis=0),
                    compute_op=mybir.AluOpType.add,
                )
                if n in dve_sq:
                    nc.vector.tensor_mul(out=junk[:], in0=tl[:], in1=tl[:])
                    nc.vector.tensor_reduce(
                        out=acc[:, n:n + 1], in_=junk[:],
                        axis=mybir.AxisListType.X, op=mybir.AluOpType.add)
                else:
                    # junk out (bf16): fp8 tiles would saturate on squares
                    nc.scalar.activation(
                        out=junk[:], in_=tl[:],
                        func=mybir.ActivationFunctionType.Square,
                        accum_out=acc[:, n:n + 1],
                    )
            nc.sync.dma_start(out=partials[:], in_=acc[:])

    nc.compile()
    return nc


def _build_dma_gather(nc, bass, mybir, TileContext, c_dt, x, idx16, centers,
                      partials, x_tiles, x_bufs, dve_sq, n_gathers):
    rows_per_gather = ROWS_PER_CORE // n_gathers
    groups_per_gather = rows_per_gather // P
    scols = rows_per_gather // 16

    with TileContext(nc) as tc:
        with (
            tc.tile_pool(name="xp", bufs=x_bufs) as xp,
            tc.tile_pool(name="cp", bufs=n_gathers) as cp,
            tc.tile_pool(name="small", bufs=1) as small,
        ):
            it = small.tile([P, ROWS_PER_CORE // 16], mybir.dt.int16)
            nc.gpsimd.dma_start(out=it[:], in_=idx16[:])
            acc = small.tile([P, ROW_GROUPS], mybir.dt.float32)
            junk = small.tile([P, F], mybir.dt.bfloat16)

            c_tiles = []
            for t in range(n_gathers):
                ct = cp.tile([P, groups_per_gather, F], c_dt, tag="ct")
                nc.gpsimd.dma_gather(
                    out_ap=ct[:],
                    in_ap=centers[:],
                    idxs_ap=it[:16, t * scols:(t + 1) * scols],
                    num_idxs=rows_per_gather,
                    num_idxs_reg=rows_per_gather,
                    elem_size=F,
                    single_packet=False,
                    queue_num=1,
                )
                c_tiles.append(ct)

            for n in range(ROW_GROUPS):
                xt = xp.tile([P, F], mybir.dt.bfloat16, tag="xt")
                nc.sync.dma_start(out=xt[:], in_=x_tiles[n])
                t, g = divmod(n, groups_per_gather)
                nc.vector.tensor_add(out=xt[:], in0=xt[:],
                                     in1=c_tiles[t][:, g, :])
                if n in dve_sq:
                    nc.vector.tensor_mul(out=junk[:], in0=xt[:], in1=xt[:])
                    nc.vector.tensor_reduce(
                        out=acc[:, n:n + 1], in_=junk[:],
                        axis=mybir.AxisListType.X, op=mybir.AluOpType.add)
                else:
                    nc.scalar.activation(
                        out=xt[:], in_=xt[:],
                        func=mybir.ActivationFunctionType.Square,
                        accum_out=acc[:, n:n + 1],
                    )
            nc.sync.dma_start(out=partials[:], in_=acc[:])

    nc.compile()
    return nc


def _make_idx16(labels_core):
    blk = np.ascontiguousarray(
        labels_core.astype(np.int16).reshape(-1, 16).T)  # [16, 64]
    return np.ascontiguousarray(np.tile(blk, (8, 1)))    # [128, 64]


def _get_program():
    key = (GATHER_MODE, CENTER_DT, X_DT, X_BUFS, tuple(DVE_SQ_TILES),
           N_GATHERS, DMA_SCRATCH)
    if key not in _CACHE:
        _CACHE[key] = _build_program(*key)
    return _CACHE[key]


def kernel(x, labels, centers, _trace=False, _trace_cores=None):
    import ml_dtypes
    from concourse.bass_utils import run_bass_kernel_spmd

    x = np.asarray(x)
    labels = np.asarray(labels)
    centers = np.asarray(centers)
    assert x.shape == (B, F) and centers.shape == (C, F)

    nc = _get_program()

    neg_x = np.ascontiguousarray((-x).astype(_np_dt(X_DT)))
    centers_q = np.ascontiguousarray(centers.astype(_np_dt(CENTER_DT)))
    labels32 = labels.astype(np.int32)

    # Exact self-term corrections (see module docstring):
    #   sum_b [||dc_lab||^2 + 2 dc_lab.c_lab] + sum_b [||dx_b||^2 + 2 dx_b.x_b]
    counts = np.bincount(labels32, minlength=C).astype(np.float64)
    c64 = centers.astype(np.float64)
    dc = centers_q.astype(np.float64) - c64
    corr_c = float(counts @ ((dc * dc).sum(axis=1) + 2.0 * (dc * c64).sum(axis=1)))
    x64 = x.astype(np.float64)
    dx = (-neg_x).astype(np.float64) - x64
    corr_x = float((dx * dx).sum() + 2.0 * (dx * x64).sum())
    correction = corr_c + corr_x

    in_maps = []
    for k in range(N_CORES):
        lo = k * ROWS_PER_CORE
        lab_k = labels32[lo:lo + ROWS_PER_CORE].reshape(ROW_GROUPS, P).T
        in_maps.append({
            "x": neg_x[lo:lo + ROWS_PER_CORE],
            "labels_t": np.ascontiguousarray(lab_k),
            "idx16": _make_idx16(labels32[lo:lo + ROWS_PER_CORE]),
            "centers": centers_q,
        })

    res = run_bass_kernel_spmd(
        nc, in_maps, list(range(N_CORES)),
        trace=_trace,
        trace_cores=_trace_cores if _trace else None,
    )
    _CACHE["last_result"] = res

    total = np.float64(0.0)
    for r in res.results:
        total += r["partials"].astype(np.float64).sum()
    loss = (total - correction) / B + (C - 1) * 1e-12
    return np.float32(loss)


# revision 21
# speedup vs baseline: 1.2336x; 1.0484x over previous
"""CenterLoss on 8 Trainium2 NeuronCores.

Math: the reference masks the full (B, C) distance matrix down to one entry
per row and clips zeros up to 1e-12, so

    loss = mean_b ||x_b - centers[labels_b]||^2 + (C-1) * 1e-12

exactly (the matched entries are chi-square-distributed around 4096 and never
touch either clip bound). No (B, C) matmul is needed — the kernel is a
row gather + fused subtract/square/reduce.

Distribution: data-parallel over the batch. Each of the 8 cores gets 1024
rows of x (negated, bf16) + labels; centers (fp8-e3m4 by default) are
replicated in each core's DRAM. Per 128-row tile the kernel
  1. DMAs the -x tile to SBUF (HWDGE),
  2. indirect-DMA-gathers centers[label] onto it with cce_op=add — the
     DMA's inline CCE adder computes (c - x) in bf16; the sign is
     irrelevant under squaring,
  3. runs one ScalarE activation(Square, accum_out) for the row sums.
Per-partition partials are summed on host in float64.

Quantization handling: with c~ = q(c), x~ = bf16(x),
  ||x~ - c~||^2 - ||x - c||^2
    = [||dc||^2 + 2 dc.c] + [||dx||^2 + 2 dx.x] - 2 dx.c - 2 dc.x - 2 dx.dc
The bracketed self-terms are computed exactly on host (per-class for c,
per-row for x) and subtracted; the remaining cross terms are zero-mean
(quantization noise independent of the other operand) and contribute only
~4e-6 relative noise across the 16.8M summed elements.
"""

import numpy as np

B = 8192
F = 2048
C = 4096
N_CORES = 8
P = 128
ROWS_PER_CORE = B // N_CORES  # 1024
ROW_GROUPS = ROWS_PER_CORE // P  # 8

# --- tunables -------------------------------------------------------------
GATHER_MODE = "indirect"  # "indirect" (CCE-fused subtract) | "dma_gather"
CENTER_DT = "fp8e3"  # "bf16" | "fp8e3" | "fp8e4"
X_DT = "bf16"        # "bf16" | "fp8e3"
X_BUFS = 8
DVE_SQ_TILES = ()    # row-groups whose square+reduce runs on VectorE
PLAIN_TILES = (0, 1, 2, 3)  # indirect mode: row-groups gathered WITHOUT the
                     # CCE add (plain fp8 gather + VectorE add) — halves those
                     # descriptors' SDMA cost; kept to the EARLY tiles so the
                     # kernel tail stays on the short CCE->ACT path
N_GATHERS = 4        # dma_gather mode: gather ops per core
DMA_SCRATCH = 65536  # SWDGE descriptor-ring bytes (default 16384)
# --------------------------------------------------------------------------

_CACHE: dict = {}


def _np_dt(name):
    import ml_dtypes
    return {"bf16": ml_dtypes.bfloat16,
            "fp8e3": ml_dtypes.float8_e3m4,
            "fp8e4": ml_dtypes.float8_e4m3}[name]


def _build_program(mode, center_dt, x_dt_name, x_bufs, dve_sq, n_gathers, scratch):
    # PLAIN_TILES read from module global (part of the cache key)
    import concourse.bacc as bacc
    import concourse.bass as bass
    import concourse.mybir as mybir
    from concourse.tile import TileContext

    c_dt = {"bf16": mybir.dt.bfloat16,
            "fp8e3": mybir.dt.float8e3,
            "fp8e4": mybir.dt.float8e4}[center_dt]
    x_dt = {"bf16": mybir.dt.bfloat16,
            "fp8e3": mybir.dt.float8e3}[x_dt_name]

    nc = bacc.Bacc("TRN2", target_bir_lowering=False, debug=False,
                   num_devices=N_CORES, dynamic_dma_scratch_size=scratch,
                   num_swdge_queues=2)
    x = nc.dram_tensor("x", [ROWS_PER_CORE, F], x_dt,
                       kind="ExternalInput")  # holds -x
    labels_t = nc.dram_tensor("labels_t", [P, ROW_GROUPS], mybir.dt.int32,
                              kind="ExternalInput")  # [p, n] = label[n*128+p]
    # dma_gather mode: [p, s] = labels[s*16 + (p%16)], the 16-partition wrap
    # replicated into all 8 gpsimd cores' partition windows.
    idx16 = nc.dram_tensor("idx16", [P, ROWS_PER_CORE // 16], mybir.dt.int16,
                           kind="ExternalInput")
    centers = nc.dram_tensor("centers", [C, F], c_dt, kind="ExternalInput")
    partials = nc.dram_tensor("partials", [P, ROW_GROUPS], mybir.dt.float32,
                              kind="ExternalOutput")

    x_tiles = x[:].rearrange("(n p) f -> n p f", p=P)

    if mode == "dma_gather":
        return _build_dma_gather(nc, bass, mybir, TileContext, c_dt, x, idx16,
                                 centers, partials, x_tiles, x_bufs, dve_sq,
                                 n_gathers)
    assert x_dt_name == "bf16" or mode == "indirect"

    with TileContext(nc) as tc:
        with (
            tc.tile_pool(name="work", bufs=x_bufs) as work,
            tc.tile_pool(name="small", bufs=1) as small,
        ):
            # SWDGE load: precedes the gathers in the Q7 queue and keeps
            # their wait off the shared HWDGE sem lanes (an HWDGE labels
            # load shares a lane with the 8th x load and stalls gather 0).
            lab = small.tile([P, ROW_GROUPS], mybir.dt.int32)
            nc.gpsimd.dma_start(out=lab[:], in_=labels_t[:])
            acc = small.tile([P, ROW_GROUPS], mybir.dt.float32)
            junk = small.tile([P, F], mybir.dt.bfloat16)
            cpool_cm = tc.tile_pool(name="cp", bufs=max(1, len(PLAIN_TILES)))
            cpool = cpool_cm.__enter__()
            for n in range(ROW_GROUPS):
                tl = work.tile([P, F], x_dt, tag="xt")
                nc.sync.dma_start(out=tl[:], in_=x_tiles[n])
                if n in PLAIN_TILES:
                    # plain gather (half the SDMA descriptor cost of the CCE
                    # RMW path) + VectorE add; keeps d in bf16.
                    ct = cpool.tile([P, F], c_dt, tag="ct")
                    nc.gpsimd.indirect_dma_start(
                        out=ct[:],
                        out_offset=None,
                        in_=centers[:],
                        in_offset=bass.IndirectOffsetOnAxis(
                            ap=lab[:, n:n + 1], axis=0),
                    )
                    nc.vector.tensor_add(out=tl[:], in0=tl[:], in1=ct[:])
                else:
                    # tl += centers[labels] via the DMA's inline CCE adder
                    # (fp8 source is cast on the way); tl becomes (c - x).
                    nc.gpsimd.indirect_dma_start(
                        out=tl[:],
                        out_offset=None,
                        in_=centers[:],
                        in_offset=bass.IndirectOffsetOnAxis(ap=lab[:, n:n + 1],
                                                            axis=0),
                        compute_op=mybir.AluOpType.add,
                    )
                if n in dve_sq:
                    nc.vector.tensor_mul(out=junk[:], in0=tl[:], in1=tl[:])
                    nc.vector.tensor_reduce(
                        out=acc[:, n:n + 1], in_=junk[:],
                        axis=mybir.AxisListType.X, op=mybir.AluOpType.add)
                else:
                    # junk out (bf16): fp8 tiles would saturate on squares
                    nc.scalar.activation(
                        out=junk[:], in_=tl[:],
                        func=mybir.ActivationFunctionType.Square,
                        accum_out=acc[:, n:n + 1],
                    )
            nc.sync.dma_start(out=partials[:], in_=acc[:])
            cpool_cm.__exit__(None, None, None)

    nc.compile()
    return nc


def _build_dma_gather(nc, bass, mybir, TileContext, c_dt, x, idx16, centers,
                      partials, x_tiles, x_bufs, dve_sq, n_gathers):
    rows_per_gather = ROWS_PER_CORE // n_gathers
    groups_per_gather = rows_per_gather // P
    scols = rows_per_gather // 16

    with TileContext(nc) as tc:
        with (
            tc.tile_pool(name="xp", bufs=x_bufs) as xp,
            tc.tile_pool(name="cp", bufs=n_gathers) as cp,
            tc.tile_pool(name="small", bufs=1) as small,
        ):
            it = small.tile([P, ROWS_PER_CORE // 16], mybir.dt.int16)
            nc.gpsimd.dma_start(out=it[:], in_=idx16[:])
            acc = small.tile([P, ROW_GROUPS], mybir.dt.float32)
            junk = small.tile([P, F], mybir.dt.bfloat16)

            c_tiles = []
            for t in range(n_gathers):
                ct = cp.tile([P, groups_per_gather, F], c_dt, tag="ct")
                nc.gpsimd.dma_gather(
                    out_ap=ct[:],
                    in_ap=centers[:],
                    idxs_ap=it[:16, t * scols:(t + 1) * scols],
                    num_idxs=rows_per_gather,
                    num_idxs_reg=rows_per_gather,
                    elem_size=F,
                    single_packet=False,
                    queue_num=1,
                )
                c_tiles.append(ct)

            for n in range(ROW_GROUPS):
                xt = xp.tile([P, F], mybir.dt.bfloat16, tag="xt")
                nc.sync.dma_start(out=xt[:], in_=x_tiles[n])
                t, g = divmod(n, groups_per_gather)
                nc.vector.tensor_add(out=xt[:], in0=xt[:],
                                     in1=c_tiles[t][:, g, :])
                if n in dve_sq:
                    nc.vector.tensor_mul(out=junk[:], in0=xt[:], in1=xt[:])
                    nc.vector.tensor_reduce(
                        out=acc[:, n:n + 1], in_=junk[:],
                        axis=mybir.AxisListType.X, op=mybir.AluOpType.add)
                else:
                    nc.scalar.activation(
                        out=xt[:], in_=xt[:],
                        func=mybir.ActivationFunctionType.Square,
                        accum_out=acc[:, n:n + 1],
                    )
            nc.sync.dma_start(out=partials[:], in_=acc[:])

    nc.compile()
    return nc


def _make_idx16(labels_core):
    blk = np.ascontiguousarray(
        labels_core.astype(np.int16).reshape(-1, 16).T)  # [16, 64]
    return np.ascontiguousarray(np.tile(blk, (8, 1)))    # [128, 64]


def _get_program():
    key = (GATHER_MODE, CENTER_DT, X_DT, X_BUFS, tuple(DVE_SQ_TILES),
           N_GATHERS, DMA_SCRATCH, tuple(PLAIN_TILES))
    if key not in _CACHE:
        _CACHE[key] = _build_program(*key[:7])
    return _CACHE[key]


def kernel(x, labels, centers, _trace=False, _trace_cores=None):
    import ml_dtypes
    from concourse.bass_utils import run_bass_kernel_spmd

    x = np.asarray(x)
    labels = np.asarray(labels)
    centers = np.asarray(centers)
    assert x.shape == (B, F) and centers.shape == (C, F)

    nc = _get_program()

    neg_x = np.ascontiguousarray((-x).astype(_np_dt(X_DT)))
    centers_q = np.ascontiguousarray(centers.astype(_np_dt(CENTER_DT)))
    labels32 = labels.astype(np.int32)

    # Exact self-term corrections (see module docstring):
    #   sum_b [||dc_lab||^2 + 2 dc_lab.c_lab] + sum_b [||dx_b||^2 + 2 dx_b.x_b]
    counts = np.bincount(labels32, minlength=C).astype(np.float64)
    c64 = centers.astype(np.float64)
    dc = centers_q.astype(np.float64) - c64
    corr_c = float(counts @ ((dc * dc).sum(axis=1) + 2.0 * (dc * c64).sum(axis=1)))
    x64 = x.astype(np.float64)
    dx = (-neg_x).astype(np.float64) - x64
    corr_x = float((dx * dx).sum() + 2.0 * (dx * x64).sum())
    correction = corr_c + corr_x

    in_maps = []
    for k in range(N_CORES):
        lo = k * ROWS_PER_CORE
        lab_k = labels32[lo:lo + ROWS_PER_CORE].reshape(ROW_GROUPS, P).T
        in_maps.append({
            "x": neg_x[lo:lo + ROWS_PER_CORE],
            "labels_t": np.ascontiguousarray(lab_k),
            "idx16": _make_idx16(labels32[lo:lo + ROWS_PER_CORE]),
            "centers": centers_q,
        })

    res = run_bass_kernel_spmd(
        nc, in_maps, list(range(N_CORES)),
        trace=_trace,
        trace_cores=_trace_cores if _trace else None,
    )
    _CACHE["last_result"] = res

    total = np.float64(0.0)
    for r in res.results:
        total += r["partials"].astype(np.float64).sum()
    loss = (total - correction) / B + (C - 1) * 1e-12
    return np.float32(loss)
